# revision 73
# baseline (speedup 1.0000x reference)
"""Trainium2 Bass kernel for CustomPatchEmbedding (ragged patch gather + two projections).

Final strategy (data-parallel over batch, 8 cores x 4 images; ~45us vs the
78us starting baseline):
  - Fine branch (bf16): images repacked on host into a sliding 16-row-block
    channel-last layout; a fine 16x16 patch is ONE contiguous 1536B run and
    one indirect DMA gathers each 128-patch group (the HW DGE consumes exactly
    one offset per destination partition; multi-offset APs are silently
    truncated — verified on HW).
  - Coarse branch (fp8 e4m3): a second sliding 64-row-block blob quantized to
    e4m3 on host; a coarse 64x64 patch is ONE contiguous 12288B run and the
    whole coarse gather is a single indirect DMA. Coarse weights are
    pre-scaled by S_W=256 and quantized to e4m3 (quarters weight traffic vs
    fp32, halves vs bf16); the epilogue rescales by 1/S_W. Coarse matmuls run
    in DoubleRow perf mode (2 k-chunks per instruction, 2x fp8 throughput).
    Measured end-to-end rel-err: 0.0128 (tolerance 2e-2).
  - Transposes: fine activations are plain bf16 PE transposes; coarse fp8
    activations are transposed as packed fp32 words (one [128,128] fp32
    transpose moves 4 fp8 k-chunks) and the quad interleave is undone inside
    the PSUM->SBUF copy, split across DVE and ACT so the two byte-granular
    half-copies run in parallel. All matmul lhsT reads are then contiguous
    (strided LDWEIGHTS costs ~40ns extra per matmul). The DMA XBAR transpose
    was tried and is poison on HW: ~250B packets clog the shared DMA engines.
  - Scheduling: a memset-fed PE warm-up absorbs the Tensor-engine p-state
    ramp while the first gather is in flight; coarse transpose/matmul tiles
    are interleaved into the fine-group sequence to fill the bubbles left by
    the serialized (~1.45us each) SWDGE gather issues; the coarse-weight
    loads ride the same GpSimd software queue behind the gathers, FIFO order
    matching consumption order.
  - Outputs are written bf16 (upcast to fp32 on host); the coarse epilogue is
    a single scalar_tensor_tensor and a single 3D-AP output DMA.

kernel(**inputs) takes the FULL unsharded inputs and returns (32, 288, 256) f32.
"""
import sys
import numpy as np

sys.path.insert(0, "/opt/trn_rl_repo")

import ml_dtypes
import concourse.bass as bass
import concourse.bacc as bacc
import concourse.mybir as mybir
import concourse.tile as tile
from concourse.bass_utils import run_bass_kernel_spmd
from contextlib import ExitStack

# Problem constants (hardcoded per spec).
B, C, H, W = 32, 3, 512, 512
FP, CP = 16, 64
NF, NCO = 256, 32
D = 256
NCORES = 8
IPC = B // NCORES              # images per core
KF = C * FP * FP               # 768   fine features
KC = C * CP * CP               # 12288 coarse features
P = 128
GF = IPC * 2                   # 8 fine groups of 128 patches per core
S_W = 256.0                    # coarse-weight pre-scale before e4m3 quantization

RUN_F = FP * FP * C            # 768 elements per fine gather run (whole patch)
BLK_F = W * FP * C             # fine blob stride per y-block
NROW_F = H - FP + 1            # 497 y-blocks stored
IMG_F = NROW_F * BLK_F
BLK_C = W * CP * C             # coarse blob stride per y-block
NROW_C = H - CP + 1            # 449 y-blocks stored
IMG_C = NROW_C * BLK_C

NQF = KF // (2 * P)            # 3 fine fp32-transpose blocks (2 bf16 chunks each)
NQC = KC // (4 * P)            # 24 coarse fp32-transpose blocks (4 fp8 chunks each)
NTC = NQC // 3                 # 8 coarse transpose tiles (3 blocks per tile)

FDT = mybir.dt.float32
RDT = mybir.dt.float32    # float32r transposes are 1.5 cyc/row vs 2.0 but fail BIR verification
BDT = mybir.dt.bfloat16
F8 = mybir.dt.float8e4
IDT = mybir.dt.int32
BF16 = ml_dtypes.bfloat16
E3M4 = ml_dtypes.float8_e4m3


def _emit(nc, tc, t):
    """Emit the per-core Tile program. `t` maps tensor name -> dram handle."""
    with ExitStack() as ctx:
        const = ctx.enter_context(tc.tile_pool(name="const", bufs=1))
        gf_pool = ctx.enter_context(tc.tile_pool(name="gf", bufs=GF))
        wc_pool = ctx.enter_context(tc.tile_pool(name="wc", bufs=4))
        lt_f = ctx.enter_context(tc.tile_pool(name="lt_f", bufs=3))
        lt_c = ctx.enter_context(tc.tile_pool(name="lt_c", bufs=3))
        ob_pool = ctx.enter_context(tc.tile_pool(name="ob", bufs=3))
        ps_tp = ctx.enter_context(tc.tile_pool(name="ps_tp", bufs=4, space="PSUM"))
        ps_f = ctx.enter_context(tc.tile_pool(name="ps_f", bufs=2, space="PSUM"))
        ps_c = ctx.enter_context(tc.tile_pool(name="ps_c", bufs=1, space="PSUM"))

        # --- offsets first so gathers can start immediately. The sync queue
        # carries ONLY what gates the gather/warm-up critical path (the tile
        # framework's semaphore waits are per-queue, so a big load on the same
        # queue would delay the first PE op); everything else goes on the
        # scalar queue.
        fidx = const.tile([P, GF], IDT)
        nc.sync.dma_start(fidx[:], t["fidx"][:])
        identb = const.tile([P, P], BDT)
        nc.sync.dma_start(identb[:], t["identb"][:])
        cidx = const.tile([P, 2], IDT)
        nc.sync.dma_start(cidx[:], t["cidx"][:])
        ident = const.tile([P, P], RDT)
        nc.sync.dma_start(ident[:], t["ident"][:])

        # --- PE warm-up: the Tensor engine ramps to full clock only after
        # ~3us of continuous work. Transpose garbage from a memset tile
        # (engine-to-engine semaphores are far cheaper than DMA completion
        # semaphores, so this starts ~8.5us) until the first gather lands,
        # so the real pipeline starts at full clock.
        ps_w = ctx.enter_context(tc.tile_pool(name="ps_w", bufs=1, space="PSUM"))
        warm_src = const.tile([P, P], BDT)
        nc.vector.memset(warm_src[:], 0)
        warm = ps_w.tile([P, P], FDT)
        NWARM = 75   # bridge until the first gather lands (~15us) so the PE
        for i in range(NWARM):  # never idles and re-ramps its p-state
            nc.tensor.matmul(out=warm[:], lhsT=warm_src[:], rhs=warm_src[:],
                             start=(i == 0), stop=(i == NWARM - 1))
        wf = const.tile([P, (KF // P) * D], BDT)
        nc.sync.dma_start(wf[:], t["wf2"][:])
        bias_f = const.tile([P, D], FDT)
        nc.sync.dma_start(bias_f[:], t["bias_f"][:])
        bias_c = const.tile([P, D], FDT)
        nc.sync.dma_start(bias_c[:], t["bias_c"][:])

        # --- gathers: the 8 fine groups are the latency-critical stream and
        # go first; the coarse gather follows; the coarse weight loads are
        # issued on the same software queue AFTER them (FIFO deprioritization).
        gfs = []
        gc = const.tile([P, KC], F8)

        def emit_gather_f(g):
            nc.gpsimd.indirect_dma_start(
                out=gfs[g][:], out_offset=None, in_=t["imgs16"][:],
                in_offset=bass.IndirectOffsetOnAxis(ap=fidx[:, g:g + 1], axis=0),
            )

        for g in range(GF):
            gfs.append(gf_pool.tile([P, RUN_F], BDT, tag="gf", name=f"gf{g}"))
        # wc is 3 loads: 24+24+48 blocks. The last (48-block) load merges two
        # 24-block loads to save one ~1.2us GpSimd SWDGE issue slot; it still
        # lands (~29us) before coarse tiles c4-c7 consume blocks 48-95.
        wct = [wc_pool.tile([P, 24 * D], F8, tag="wc", name="wcA"),
               wc_pool.tile([P, 24 * D], F8, tag="wc", name="wcB"),
               wc_pool.tile([P, 48 * D], F8, tag="wc2", name="wcC", bufs=1)]
        wc = [wct[0], wct[1], wct[2], wct[2]]
        wc_col0 = [0, 0, 0, 24 * D]

        def emit_wc_load(eng, s):
            w = 48 if s == 2 else 24
            eng.dma_start(wct[s][:], t["wc2"][:, (24 * s) * D:(24 * s + w) * D])

        # Queue-0 FIFO order matched to the interleaved compute schedule below:
        # fine g0-g3 first (PE start), then the coarse gather in HALVES (the
        # subtile dep tracker lets coarse tiles 0-3 start on the first half)
        # interleaved with the remaining fine groups, then the weight tiles.
        for g in range(4):
            emit_gather_f(g)
        nc.gpsimd.indirect_dma_start(
            out=gc[:, :KC // 2], out_offset=None, in_=t["imgs8c"][:],
            in_offset=bass.IndirectOffsetOnAxis(ap=cidx[:, 0:1], axis=0),
        )
        emit_wc_load(nc.gpsimd, 0)
        emit_gather_f(4)
        emit_gather_f(5)
        nc.gpsimd.indirect_dma_start(
            out=gc[:, KC // 2:], out_offset=None, in_=t["imgs8c"][:],
            in_offset=bass.IndirectOffsetOnAxis(ap=cidx[:, 1:2], axis=0),
        )
        emit_gather_f(6)
        emit_gather_f(7)
        for s in range(1, 3):
            emit_wc_load(nc.gpsimd, s)

        out = t["out"]
        psum_c = ps_c.tile([P, D], FDT)

        # --- stages: T (PE transposes + DVE copy), M (matmuls) ---
        # Fine transposes are plain bf16 (6 per group); coarse transposes are
        # quad-packed fp32 views (4 fp8 chunks per [128,128] transpose) whose
        # interleave is undone in the DVE copy, so every matmul reads a
        # CONTIGUOUS lhsT (strided LDWEIGHTS costs ~40ns extra per matmul).
        def fine_T(g):
            tp = ps_tp.tile([P, KF], BDT, tag="tp")
            for j in range(KF // P):
                nc.tensor.transpose(
                    out=tp[:, j * P:(j + 1) * P],
                    in_=gfs[g][:, j * P:(j + 1) * P],
                    identity=identb[:],
                )
            lt = lt_f.tile([P, NQF * P], RDT, tag="ltf")
            nc.vector.tensor_copy(lt[:], tp[:].bitcast(RDT))
            return lt

        def fine_M(g, lt):
            psum = ps_f.tile([P, D], FDT, tag="psf")
            ltb = lt[:].bitcast(BDT)                   # [128, 768]
            for j in range(KF // P):
                nc.tensor.matmul(
                    out=psum[:],
                    lhsT=ltb[:, j * P:(j + 1) * P],
                    rhs=wf[:, j * D:(j + 1) * D],
                    start=(j == 0), stop=(j == KF // P - 1),
                )
            ob = ob_pool.tile([P, D], BDT, tag="ob")
            nc.vector.tensor_tensor(
                out=ob[:], in0=psum[:], in1=bias_f[:], op=mybir.AluOpType.add
            )
            b_img, hh = divmod(g, 2)
            row0 = b_img * (NF + NCO) + hh * P
            nc.scalar.dma_start(out[row0:row0 + P, :], ob[:])

        def coarse_T(tt):
            gc32 = gc[:].bitcast(RDT)                  # [128, 3072]
            tp = ps_tp.tile([P, 3 * P], RDT, tag="tp")
            for q in range(3):
                j = 3 * tt + q
                nc.tensor.transpose(
                    out=tp[:, q * P:(q + 1) * P],
                    in_=gc32[:, j * P:(j + 1) * P],
                    identity=ident[:],
                )
            lt = lt_c.tile([P, 3 * P], RDT, tag="ltc")
            # de-interleave the quad packing during the PSUM->SBUF copy:
            # tp8 col 512q + 4p + b  ->  lt8 col 512q + 128b + p.
            # Split by quad-parity across DVE and ACT so the two byte-granular
            # half-copies run in parallel instead of 1.75us serial on DVE.
            tp8 = tp[:].bitcast(F8).rearrange("i (q p b) -> i q b p", q=3, p=P, b=4)
            lt8o = lt[:].bitcast(F8).rearrange("i (q b p) -> i q b p", q=3, b=4, p=P)
            nc.vector.tensor_copy(lt8o[:, :, 0:2, :], tp8[:, :, 0:2, :])
            nc.scalar.activation(
                lt8o[:, :, 2:4, :], tp8[:, :, 2:4, :],
                mybir.ActivationFunctionType.Copy)
            return lt

        def coarse_M(tt, lt):
            # e4m3 DoubleRow: one matmul consumes 2 consecutive k-chunks
            # (lhsT/rhs get a middle k-subtile dim of 2).
            lt8 = lt[:].bitcast(F8)                    # [128, 1536]
            for q in range(3):
                j = 3 * tt + q
                for b_par in range(0, 4, 2):
                    blk = 4 * j + b_par
                    nc.tensor.matmul(
                        out=psum_c[:],
                        lhsT=lt8[:, (4 * q + b_par) * P:(4 * q + b_par + 2) * P]
                        .rearrange("i (k m) -> i k m", k=2),
                        rhs=wc[blk // 24][:, wc_col0[blk // 24] + (blk % 24) * D:
                                           wc_col0[blk // 24] + (blk % 24 + 2) * D]
                        .rearrange("i (k n) -> i k n", k=2),
                        start=(blk == 0), stop=(blk == 4 * NQC - 2),
                        perf_mode=mybir.MatmulPerfMode.DoubleRow,
                    )
            if tt == NTC - 1:
                oc = ob_pool.tile([P, D], BDT, tag="ob")
                nc.vector.scalar_tensor_tensor(
                    out=oc[:], in0=psum_c[:], scalar=1.0 / S_W, in1=bias_c[:],
                    op0=mybir.AluOpType.mult, op1=mybir.AluOpType.add,
                )
                out3 = out[:].rearrange("(b r) d -> b r d", b=IPC)
                nc.scalar.dma_start(out3[:, NF:NF + NCO, :], oc[:])

        # --- emit with 1-stage software pipelining: T(s+1) before M(s).
        # Coarse tiles are interleaved into the fine sequence so the PE fills
        # the bubbles left by the serialized fine gather issue (~1.45us per
        # group on GpSimd SWDGE) instead of idling.
        # The last stage is a fine group: its epilogue (bias add + 1 small
        # DMA) is a shorter tail than the coarse epilogue, which now overlaps
        # f7's matmuls.
        stages = [("f", 0), ("f", 1), ("f", 2), ("f", 3),
                  ("c", 0), ("f", 4), ("c", 1), ("f", 5),
                  ("c", 2), ("f", 6), ("c", 3), ("f", 7),
                  ("c", 4), ("c", 5), ("c", 6), ("c", 7)]
        prev = None
        for kind, i in stages:
            lt = fine_T(i) if kind == "f" else coarse_T(i)
            if prev is not None:
                pk, pi, plt = prev
                (fine_M if pk == "f" else coarse_M)(pi, plt)
            prev = (kind, i, lt)
        pk, pi, plt = prev
        (fine_M if pk == "f" else coarse_M)(pi, plt)


def build(reps: int = 1):
    nc = bacc.Bacc("TRN2", target_bir_lowering=False, debug=False)
    t = {
        "imgs16": nc.dram_tensor("imgs16", [IPC * IMG_F, 1], BDT, kind="ExternalInput"),
        "imgs8c": nc.dram_tensor("imgs8c", [IPC * IMG_C, 1], F8, kind="ExternalInput"),
        "wf2": nc.dram_tensor("wf2", [P, (KF // P) * D], BDT, kind="ExternalInput"),
        "wc2": nc.dram_tensor("wc2", [P, (KC // P) * D], F8, kind="ExternalInput"),
        "bias_f": nc.dram_tensor("bias_f", [P, D], FDT, kind="ExternalInput"),
        "bias_c": nc.dram_tensor("bias_c", [P, D], FDT, kind="ExternalInput"),
        "ident": nc.dram_tensor("ident", [P, P], RDT, kind="ExternalInput"),
        "identb": nc.dram_tensor("identb", [P, P], BDT, kind="ExternalInput"),
        "fidx": nc.dram_tensor("fidx", [P, GF], IDT, kind="ExternalInput"),
        "cidx": nc.dram_tensor("cidx", [P, 2], IDT, kind="ExternalInput"),
        "out": nc.dram_tensor("out", [IPC * (NF + NCO), D], BDT, kind="ExternalOutput"),
    }
    with tile.TileContext(nc) as tc:
        for _ in range(reps):
            _emit(nc, tc, t)
    nc.compile()
    return nc


def repack_fine(images):
    """[b, C, H, W] f32 -> sliding 16-row-block channel-last bf16 blob.

    blk[b, y, x, dy, c] = images[b, c, y+dy, x], y in [0, H-16]."""
    cl = np.ascontiguousarray(images.transpose(0, 2, 3, 1)).astype(BF16)
    sw = np.lib.stride_tricks.sliding_window_view(cl, FP, axis=1)  # [b,497,x,c,dy]
    return np.ascontiguousarray(sw.transpose(0, 1, 2, 4, 3))


def repack_coarse(images):
    """[b, C, H, W] f32 -> sliding 64-row-block channel-last e3m4 blob."""
    cl = np.ascontiguousarray(images.transpose(0, 2, 3, 1)).astype(E3M4)
    sw = np.lib.stride_tricks.sliding_window_view(cl, CP, axis=1)  # [b,449,x,c,dy]
    return np.ascontiguousarray(sw.transpose(0, 1, 2, 4, 3))


def host_indices(fine_xy, coarse_xy):
    """Element offsets into the per-core blobs (one per gather run)."""
    base_f = fine_xy[:, :, 1] * BLK_F + fine_xy[:, :, 0] * (FP * C) \
        + (np.arange(IPC) * IMG_F)[:, None]                        # [IPC, NF]
    fidx = base_f.reshape(GF, P).T                                 # [P, GF]
    base_c = coarse_xy[:, :, 1] * BLK_C + coarse_xy[:, :, 0] * (CP * C) \
        + (np.arange(IPC) * IMG_C)[:, None]                        # [IPC, NCO]
    cidx = base_c.reshape(P, 1)
    cidx = np.concatenate([cidx, cidx + KC // 2], axis=1)          # [P, 2] halves
    return (np.ascontiguousarray(fidx.astype(np.int32)),
            np.ascontiguousarray(cidx.astype(np.int32)))


def feat_perm(patch):
    """Gather order (dx, dy, c) -> original (c, dy, dx) column index."""
    dx, dy, c = np.meshgrid(
        np.arange(patch), np.arange(patch), np.arange(C), indexing="ij"
    )
    return (c * (patch * patch) + dy * patch + dx).reshape(-1)


def swizzle_w_interleave(wg, stride):
    """[K, D] gather-order weights -> [128, (K//128)*D], rows interleaved so
    block (j, b) holds rows (128*stride)*j + stride*i + b (i = partition)."""
    K = wg.shape[0]
    blocks = []
    for j in range(K // (P * stride)):
        for b in range(stride):
            blocks.append(wg[P * stride * j + stride * np.arange(P) + b])
    return np.ascontiguousarray(
        np.stack(blocks, axis=1).reshape(P, (K // P) * D)
    )


def make_in_maps(images, W_fine, b_fine, W_coarse, b_coarse, fine_xy, coarse_xy):
    images = np.asarray(images, dtype=np.float32)
    fine_xy = np.asarray(fine_xy, dtype=np.int64)
    coarse_xy = np.asarray(coarse_xy, dtype=np.int64)
    blob_f = repack_fine(images)
    blob_c = repack_coarse(images)
    wf2 = swizzle_w_interleave(
        np.asarray(W_fine, np.float32).T[feat_perm(FP)].astype(BF16), 1)
    wc2 = swizzle_w_interleave(
        (np.asarray(W_coarse, np.float32).T[feat_perm(CP)] * S_W).astype(E3M4), 4)
    bias_f = np.ascontiguousarray(
        np.repeat(np.asarray(b_fine, np.float32)[None, :], P, axis=0))
    bias_c = np.ascontiguousarray(
        np.repeat(np.asarray(b_coarse, np.float32)[None, :], P, axis=0))
    ident = np.eye(P, dtype=np.float32)
    identb = np.eye(P, dtype=BF16)
    in_maps = []
    for c in range(NCORES):
        sl = slice(c * IPC, (c + 1) * IPC)
        fidx, cidx = host_indices(fine_xy[sl], coarse_xy[sl])
        in_maps.append({
            "imgs16": blob_f[sl].reshape(IPC * IMG_F, 1),
            "imgs8c": blob_c[sl].reshape(IPC * IMG_C, 1),
            "wf2": wf2, "wc2": wc2,
            "bias_f": bias_f, "bias_c": bias_c, "ident": ident, "identb": identb,
            "fidx": fidx, "cidx": cidx,
        })
    return in_maps


_NC_CACHE = []


def _get_nc():
    if not _NC_CACHE:
        _NC_CACHE.append(build())
    return _NC_CACHE[0]


def run(inputs: dict, trace: bool = False):
    nc = _get_nc()
    in_maps = make_in_maps(**inputs)
    res = run_bass_kernel_spmd(nc, in_maps, list(range(NCORES)), trace=trace)
    outs = [
        np.asarray(res.results[c]["out"]).astype(np.float32).reshape(IPC, NF + NCO, D)
        for c in range(NCORES)
    ]
    return np.concatenate(outs, axis=0), res


def kernel(**inputs) -> np.ndarray:
    out, _ = run(inputs, trace=False)
    return out


# revision 75
# speedup vs baseline: 1.0958x; 1.0958x over previous
"""Trainium2 Bass kernel for CustomPatchEmbedding (ragged patch gather + two projections).

Final strategy (data-parallel over batch, 8 cores x 4 images; ~45us vs the
78us starting baseline):
  - Fine branch (bf16): images repacked on host into a sliding 16-row-block
    channel-last layout; a fine 16x16 patch is ONE contiguous 1536B run and
    one indirect DMA gathers each 128-patch group (the HW DGE consumes exactly
    one offset per destination partition; multi-offset APs are silently
    truncated — verified on HW).
  - Coarse branch (fp8 e4m3): a second sliding 64-row-block blob quantized to
    e4m3 on host; a coarse 64x64 patch is ONE contiguous 12288B run and the
    whole coarse gather is a single indirect DMA. Coarse weights are
    pre-scaled by S_W=256 and quantized to e4m3 (quarters weight traffic vs
    fp32, halves vs bf16); the epilogue rescales by 1/S_W. Coarse matmuls run
    in DoubleRow perf mode (2 k-chunks per instruction, 2x fp8 throughput).
    Measured end-to-end rel-err: 0.0128 (tolerance 2e-2).
  - Transposes: fine activations are plain bf16 PE transposes; coarse fp8
    activations are transposed as packed fp32 words (one [128,128] fp32
    transpose moves 4 fp8 k-chunks) and the quad interleave is undone inside
    the PSUM->SBUF copy, split across DVE and ACT so the two byte-granular
    half-copies run in parallel. All matmul lhsT reads are then contiguous
    (strided LDWEIGHTS costs ~40ns extra per matmul). The DMA XBAR transpose
    was tried and is poison on HW: ~250B packets clog the shared DMA engines.
  - Scheduling: a memset-fed PE warm-up absorbs the Tensor-engine p-state
    ramp while the first gather is in flight; coarse transpose/matmul tiles
    are interleaved into the fine-group sequence to fill the bubbles left by
    the serialized (~1.45us each) SWDGE gather issues; the coarse-weight
    loads ride the same GpSimd software queue behind the gathers, FIFO order
    matching consumption order.
  - Outputs are written bf16 (upcast to fp32 on host); the coarse epilogue is
    a single scalar_tensor_tensor and a single 3D-AP output DMA.

kernel(**inputs) takes the FULL unsharded inputs and returns (32, 288, 256) f32.
"""
import sys
import numpy as np

sys.path.insert(0, "/opt/trn_rl_repo")

import ml_dtypes
import concourse.bass as bass
import concourse.bacc as bacc
import concourse.mybir as mybir
import concourse.tile as tile
from concourse.bass_utils import run_bass_kernel_spmd
from contextlib import ExitStack

# Problem constants (hardcoded per spec).
B, C, H, W = 32, 3, 512, 512
FP, CP = 16, 64
NF, NCO = 256, 32
D = 256
NCORES = 8
IPC = B // NCORES              # images per core
KF = C * FP * FP               # 768   fine features
KC = C * CP * CP               # 12288 coarse features
P = 128
GF = IPC * 2                   # 8 fine groups of 128 patches per core
S_W = 256.0                    # coarse-weight pre-scale before e4m3 quantization

RUN_F = FP * FP * C            # 768 elements per fine gather run (whole patch)
BLK_F = W * FP * C             # fine blob stride per y-block
NROW_F = H - FP + 1            # 497 y-blocks stored
IMG_F = NROW_F * BLK_F
BLK_C = W * CP * C             # coarse blob stride per y-block
NROW_C = H - CP + 1            # 449 y-blocks stored
IMG_C = NROW_C * BLK_C

NQF = KF // (2 * P)            # 3 fine fp32-transpose blocks (2 bf16 chunks each)
NQC = KC // (4 * P)            # 24 coarse fp32-transpose blocks (4 fp8 chunks each)
NTC = NQC // 3                 # 8 coarse transpose tiles (3 blocks per tile)

FDT = mybir.dt.float32
RDT = mybir.dt.float32    # float32r transposes are 1.5 cyc/row vs 2.0 but fail BIR verification
BDT = mybir.dt.bfloat16
F8 = mybir.dt.float8e4
IDT = mybir.dt.int32
BF16 = ml_dtypes.bfloat16
E3M4 = ml_dtypes.float8_e4m3


def _emit(nc, tc, t):
    """Emit the per-core Tile program. `t` maps tensor name -> dram handle."""
    with ExitStack() as ctx:
        const = ctx.enter_context(tc.tile_pool(name="const", bufs=1))
        gf_pool = ctx.enter_context(tc.tile_pool(name="gf", bufs=GF))
        wc_pool = ctx.enter_context(tc.tile_pool(name="wc", bufs=4))
        lt_f = ctx.enter_context(tc.tile_pool(name="lt_f", bufs=4))
        lt_c = ctx.enter_context(tc.tile_pool(name="lt_c", bufs=4))
        ob_pool = ctx.enter_context(tc.tile_pool(name="ob", bufs=4))
        ps_tp = ctx.enter_context(tc.tile_pool(name="ps_tp", bufs=4, space="PSUM"))
        ps_f = ctx.enter_context(tc.tile_pool(name="ps_f", bufs=2, space="PSUM"))
        ps_c = ctx.enter_context(tc.tile_pool(name="ps_c", bufs=1, space="PSUM"))

        # --- offsets first so gathers can start immediately. The sync queue
        # carries ONLY what gates the gather/warm-up critical path (the tile
        # framework's semaphore waits are per-queue, so a big load on the same
        # queue would delay the first PE op); everything else goes on the
        # scalar queue.
        fidx = const.tile([P, GF], IDT)
        nc.sync.dma_start(fidx[:], t["fidx"][:])
        identb = const.tile([P, P], BDT)
        nc.sync.dma_start(identb[:], t["identb"][:])
        cidx = const.tile([P, 2], IDT)
        nc.sync.dma_start(cidx[:], t["cidx"][:])
        ident = const.tile([P, P], RDT)
        nc.sync.dma_start(ident[:], t["ident"][:])

        # --- PE warm-up: the Tensor engine ramps to full clock only after
        # ~3us of continuous work. Transpose garbage from a memset tile
        # (engine-to-engine semaphores are far cheaper than DMA completion
        # semaphores, so this starts ~8.5us) until the first gather lands,
        # so the real pipeline starts at full clock.
        ps_w = ctx.enter_context(tc.tile_pool(name="ps_w", bufs=1, space="PSUM"))
        warm_src = const.tile([P, P], BDT)
        nc.vector.memset(warm_src[:], 0)
        warm = ps_w.tile([P, P], FDT)
        NWARM = 60   # bridge until the first gather lands (~15us) so the PE
        for i in range(NWARM):  # never idles and re-ramps its p-state
            nc.tensor.matmul(out=warm[:], lhsT=warm_src[:], rhs=warm_src[:],
                             start=(i == 0), stop=(i == NWARM - 1))
        wf = const.tile([P, (KF // P) * D], BDT)
        nc.sync.dma_start(wf[:], t["wf2"][:])
        bias_f = const.tile([P, D], FDT)
        nc.sync.dma_start(bias_f[:], t["bias_f"][:])
        bias_c = const.tile([P, D], FDT)
        nc.sync.dma_start(bias_c[:], t["bias_c"][:])

        # --- gathers: the 8 fine groups are the latency-critical stream and
        # go first; the coarse gather follows; the coarse weight loads are
        # issued on the same software queue AFTER them (FIFO deprioritization).
        gfs = []
        gc = const.tile([P, KC], F8)

        def emit_gather_f(g):
            nc.gpsimd.indirect_dma_start(
                out=gfs[g][:], out_offset=None, in_=t["imgs16"][:],
                in_offset=bass.IndirectOffsetOnAxis(ap=fidx[:, g:g + 1], axis=0),
            )

        for g in range(GF):
            gfs.append(gf_pool.tile([P, RUN_F], BDT, tag="gf", name=f"gf{g}"))
        wc = [wc_pool.tile([P, 24 * D], F8, tag="wc", name=f"wc{s}")
              for s in range(4)]

        def emit_wc_load(eng, s):
            eng.dma_start(wc[s][:], t["wc2"][:, s * 24 * D:(s + 1) * 24 * D])

        # Queue-0 FIFO order matched to the interleaved compute schedule below:
        # fine g0-g3 first (PE start), then the coarse gather in HALVES (the
        # subtile dep tracker lets coarse tiles 0-3 start on the first half)
        # interleaved with the remaining fine groups, then the weight tiles.
        for g in range(4):
            emit_gather_f(g)
        nc.gpsimd.indirect_dma_start(
            out=gc[:, :KC // 2], out_offset=None, in_=t["imgs8c"][:],
            in_offset=bass.IndirectOffsetOnAxis(ap=cidx[:, 0:1], axis=0),
        )
        emit_wc_load(nc.gpsimd, 0)
        emit_gather_f(4)
        emit_gather_f(5)
        nc.gpsimd.indirect_dma_start(
            out=gc[:, KC // 2:], out_offset=None, in_=t["imgs8c"][:],
            in_offset=bass.IndirectOffsetOnAxis(ap=cidx[:, 1:2], axis=0),
        )
        emit_gather_f(6)
        emit_gather_f(7)
        for s in range(1, 4):
            emit_wc_load(nc.gpsimd, s)

        out = t["out"]
        psum_c = ps_c.tile([P, D], FDT)

        # --- stages: T (PE transposes + DVE copy), M (matmuls) ---
        # Fine transposes are plain bf16 (6 per group); coarse transposes are
        # quad-packed fp32 views (4 fp8 chunks per [128,128] transpose) whose
        # interleave is undone in the DVE copy, so every matmul reads a
        # CONTIGUOUS lhsT (strided LDWEIGHTS costs ~40ns extra per matmul).
        def fine_T(g):
            tp = ps_tp.tile([P, KF], BDT, tag="tp")
            for j in range(KF // P):
                nc.tensor.transpose(
                    out=tp[:, j * P:(j + 1) * P],
                    in_=gfs[g][:, j * P:(j + 1) * P],
                    identity=identb[:],
                )
            lt = lt_f.tile([P, NQF * P], RDT, tag="ltf")
            nc.vector.tensor_copy(lt[:], tp[:].bitcast(RDT))
            return lt

        def fine_M(g, lt):
            psum = ps_f.tile([P, D], FDT, tag="psf")
            ltb = lt[:].bitcast(BDT)                   # [128, 768]
            for j in range(KF // P):
                nc.tensor.matmul(
                    out=psum[:],
                    lhsT=ltb[:, j * P:(j + 1) * P],
                    rhs=wf[:, j * D:(j + 1) * D],
                    start=(j == 0), stop=(j == KF // P - 1),
                )
            ob = ob_pool.tile([P, D], BDT, tag="ob")
            nc.vector.tensor_tensor(
                out=ob[:], in0=psum[:], in1=bias_f[:], op=mybir.AluOpType.add
            )
            b_img, hh = divmod(g, 2)
            row0 = b_img * (NF + NCO) + hh * P
            nc.scalar.dma_start(out[row0:row0 + P, :], ob[:])

        def coarse_T(tt):
            gc32 = gc[:].bitcast(RDT)                  # [128, 3072]
            tp = ps_tp.tile([P, 3 * P], RDT, tag="tp")
            for q in range(3):
                j = 3 * tt + q
                nc.tensor.transpose(
                    out=tp[:, q * P:(q + 1) * P],
                    in_=gc32[:, j * P:(j + 1) * P],
                    identity=ident[:],
                )
            lt = lt_c.tile([P, 3 * P], RDT, tag="ltc")
            # de-interleave the quad packing during the PSUM->SBUF copy:
            # tp8 col 512q + 4p + b  ->  lt8 col 512q + 128b + p.
            # Split by quad-parity across DVE and ACT so the two byte-granular
            # half-copies run in parallel instead of 1.75us serial on DVE.
            tp8 = tp[:].bitcast(F8).rearrange("i (q p b) -> i q b p", q=3, p=P, b=4)
            lt8o = lt[:].bitcast(F8).rearrange("i (q b p) -> i q b p", q=3, b=4, p=P)
            nc.vector.tensor_copy(lt8o[:, :, 0:2, :], tp8[:, :, 0:2, :])
            nc.scalar.activation(
                lt8o[:, :, 2:4, :], tp8[:, :, 2:4, :],
                mybir.ActivationFunctionType.Copy)
            return lt

        def coarse_M(tt, lt):
            # e4m3 DoubleRow: one matmul consumes 2 consecutive k-chunks
            # (lhsT/rhs get a middle k-subtile dim of 2).
            lt8 = lt[:].bitcast(F8)                    # [128, 1536]
            for q in range(3):
                j = 3 * tt + q
                for b_par in range(0, 4, 2):
                    blk = 4 * j + b_par
                    nc.tensor.matmul(
                        out=psum_c[:],
                        lhsT=lt8[:, (4 * q + b_par) * P:(4 * q + b_par + 2) * P]
                        .rearrange("i (k m) -> i k m", k=2),
                        rhs=wc[blk // 24][:, (blk % 24) * D:(blk % 24 + 2) * D]
                        .rearrange("i (k n) -> i k n", k=2),
                        start=(blk == 0), stop=(blk == 4 * NQC - 2),
                        perf_mode=mybir.MatmulPerfMode.DoubleRow,
                    )
            if tt == NTC - 1:
                oc = ob_pool.tile([P, D], BDT, tag="ob")
                nc.vector.scalar_tensor_tensor(
                    out=oc[:], in0=psum_c[:], scalar=1.0 / S_W, in1=bias_c[:],
                    op0=mybir.AluOpType.mult, op1=mybir.AluOpType.add,
                )
                out3 = out[:].rearrange("(b r) d -> b r d", b=IPC)
                nc.scalar.dma_start(out3[:, NF:NF + NCO, :], oc[:])

        # --- emit with 1-stage software pipelining: T(s+1) before M(s).
        # Coarse tiles are interleaved into the fine sequence so the PE fills
        # the bubbles left by the serialized fine gather issue (~1.45us per
        # group on GpSimd SWDGE) instead of idling.
        # The last stage is a fine group: its epilogue (bias add + 1 small
        # DMA) is a shorter tail than the coarse epilogue, which now overlaps
        # f7's matmuls.
        stages = [("f", 0), ("f", 1), ("f", 2), ("f", 3),
                  ("c", 0), ("f", 4), ("c", 1), ("f", 5),
                  ("c", 2), ("f", 6), ("c", 3), ("f", 7),
                  ("c", 4), ("c", 5), ("c", 6), ("c", 7)]
        prev = None
        for kind, i in stages:
            lt = fine_T(i) if kind == "f" else coarse_T(i)
            if prev is not None:
                pk, pi, plt = prev
                (fine_M if pk == "f" else coarse_M)(pi, plt)
            prev = (kind, i, lt)
        pk, pi, plt = prev
        (fine_M if pk == "f" else coarse_M)(pi, plt)


def build(reps: int = 1):
    nc = bacc.Bacc("TRN2", target_bir_lowering=False, debug=False)
    t = {
        "imgs16": nc.dram_tensor("imgs16", [IPC * IMG_F, 1], BDT, kind="ExternalInput"),
        "imgs8c": nc.dram_tensor("imgs8c", [IPC * IMG_C, 1], F8, kind="ExternalInput"),
        "wf2": nc.dram_tensor("wf2", [P, (KF // P) * D], BDT, kind="ExternalInput"),
        "wc2": nc.dram_tensor("wc2", [P, (KC // P) * D], F8, kind="ExternalInput"),
        "bias_f": nc.dram_tensor("bias_f", [P, D], FDT, kind="ExternalInput"),
        "bias_c": nc.dram_tensor("bias_c", [P, D], FDT, kind="ExternalInput"),
        "ident": nc.dram_tensor("ident", [P, P], RDT, kind="ExternalInput"),
        "identb": nc.dram_tensor("identb", [P, P], BDT, kind="ExternalInput"),
        "fidx": nc.dram_tensor("fidx", [P, GF], IDT, kind="ExternalInput"),
        "cidx": nc.dram_tensor("cidx", [P, 2], IDT, kind="ExternalInput"),
        "out": nc.dram_tensor("out", [IPC * (NF + NCO), D], BDT, kind="ExternalOutput"),
    }
    with tile.TileContext(nc) as tc:
        for _ in range(reps):
            _emit(nc, tc, t)
    nc.compile()
    return nc


def repack_fine(images):
    """[b, C, H, W] f32 -> sliding 16-row-block channel-last bf16 blob.

    blk[b, y, x, dy, c] = images[b, c, y+dy, x], y in [0, H-16]."""
    cl = np.ascontiguousarray(images.transpose(0, 2, 3, 1)).astype(BF16)
    sw = np.lib.stride_tricks.sliding_window_view(cl, FP, axis=1)  # [b,497,x,c,dy]
    return np.ascontiguousarray(sw.transpose(0, 1, 2, 4, 3))


def repack_coarse(images):
    """[b, C, H, W] f32 -> sliding 64-row-block channel-last e3m4 blob."""
    cl = np.ascontiguousarray(images.transpose(0, 2, 3, 1)).astype(E3M4)
    sw = np.lib.stride_tricks.sliding_window_view(cl, CP, axis=1)  # [b,449,x,c,dy]
    return np.ascontiguousarray(sw.transpose(0, 1, 2, 4, 3))


def host_indices(fine_xy, coarse_xy):
    """Element offsets into the per-core blobs (one per gather run)."""
    base_f = fine_xy[:, :, 1] * BLK_F + fine_xy[:, :, 0] * (FP * C) \
        + (np.arange(IPC) * IMG_F)[:, None]                        # [IPC, NF]
    fidx = base_f.reshape(GF, P).T                                 # [P, GF]
    base_c = coarse_xy[:, :, 1] * BLK_C + coarse_xy[:, :, 0] * (CP * C) \
        + (np.arange(IPC) * IMG_C)[:, None]                        # [IPC, NCO]
    cidx = base_c.reshape(P, 1)
    cidx = np.concatenate([cidx, cidx + KC // 2], axis=1)          # [P, 2] halves
    return (np.ascontiguousarray(fidx.astype(np.int32)),
            np.ascontiguousarray(cidx.astype(np.int32)))


def feat_perm(patch):
    """Gather order (dx, dy, c) -> original (c, dy, dx) column index."""
    dx, dy, c = np.meshgrid(
        np.arange(patch), np.arange(patch), np.arange(C), indexing="ij"
    )
    return (c * (patch * patch) + dy * patch + dx).reshape(-1)


def swizzle_w_interleave(wg, stride):
    """[K, D] gather-order weights -> [128, (K//128)*D], rows interleaved so
    block (j, b) holds rows (128*stride)*j + stride*i + b (i = partition)."""
    K = wg.shape[0]
    blocks = []
    for j in range(K // (P * stride)):
        for b in range(stride):
            blocks.append(wg[P * stride * j + stride * np.arange(P) + b])
    return np.ascontiguousarray(
        np.stack(blocks, axis=1).reshape(P, (K // P) * D)
    )


def make_in_maps(images, W_fine, b_fine, W_coarse, b_coarse, fine_xy, coarse_xy):
    images = np.asarray(images, dtype=np.float32)
    fine_xy = np.asarray(fine_xy, dtype=np.int64)
    coarse_xy = np.asarray(coarse_xy, dtype=np.int64)
    blob_f = repack_fine(images)
    blob_c = repack_coarse(images)
    wf2 = swizzle_w_interleave(
        np.asarray(W_fine, np.float32).T[feat_perm(FP)].astype(BF16), 1)
    wc2 = swizzle_w_interleave(
        (np.asarray(W_coarse, np.float32).T[feat_perm(CP)] * S_W).astype(E3M4), 4)
    bias_f = np.ascontiguousarray(
        np.repeat(np.asarray(b_fine, np.float32)[None, :], P, axis=0))
    bias_c = np.ascontiguousarray(
        np.repeat(np.asarray(b_coarse, np.float32)[None, :], P, axis=0))
    ident = np.eye(P, dtype=np.float32)
    identb = np.eye(P, dtype=BF16)
    in_maps = []
    for c in range(NCORES):
        sl = slice(c * IPC, (c + 1) * IPC)
        fidx, cidx = host_indices(fine_xy[sl], coarse_xy[sl])
        in_maps.append({
            "imgs16": blob_f[sl].reshape(IPC * IMG_F, 1),
            "imgs8c": blob_c[sl].reshape(IPC * IMG_C, 1),
            "wf2": wf2, "wc2": wc2,
            "bias_f": bias_f, "bias_c": bias_c, "ident": ident, "identb": identb,
            "fidx": fidx, "cidx": cidx,
        })
    return in_maps


_NC_CACHE = []


def _get_nc():
    if not _NC_CACHE:
        _NC_CACHE.append(build())
    return _NC_CACHE[0]


def run(inputs: dict, trace: bool = False):
    nc = _get_nc()
    in_maps = make_in_maps(**inputs)
    res = run_bass_kernel_spmd(nc, in_maps, list(range(NCORES)), trace=trace)
    outs = [
        np.asarray(res.results[c]["out"]).astype(np.float32).reshape(IPC, NF + NCO, D)
        for c in range(NCORES)
    ]
    return np.concatenate(outs, axis=0), res


def kernel(**inputs) -> np.ndarray:
    out, _ = run(inputs, trace=False)
    return out
